# revision 19
# baseline (speedup 1.0000x reference)
"""GCN node-classifier forward on 8 Trainium2 NeuronCores.

out = log_softmax(P(selu(BN(P^2 (x W1) + b1))) W2 + b2),
P = D^-1/2 (A+I) D^-1/2.

Strategy: nodes row-sharded across 8 cores; edges partitioned by dst core and
sorted by (src-quarter, dst-window).  Per propagation step: the per-node scaled
features u = dinv*h are AllGathered into 4 quarter "slabs" (quarter-permuted
row order so each AllGather produces one contiguous slab); each core gathers
u[src] for its edges via dma_gather (int16 indices are valid within one slab)
and segment-sums them into its dst windows with one-hot matmuls accumulating
in PSUM.  The uniform (max over cores) padded schedule keeps the SPMD
instruction stream identical on every core.
"""

import sys

sys.path.insert(0, "/opt/trn_rl_repo")

import numpy as np
from ml_dtypes import bfloat16 as _BF16

import concourse.bacc as bacc
import concourse.tile as tile
from concourse import bass, mybir
from concourse.bass_utils import run_bass_kernel_spmd

F32 = mybir.dt.float32
BF16 = mybir.dt.bfloat16
I16 = mybir.dt.int16
I32 = mybir.dt.int32

G_Q7 = 96   # groups per 96-group chunk gathered via Q7 swdge dma_gather;
            # any remainder would go through the hardware indirect-DMA
            # walker (measured slower per 128-row call: its per-instruction
            # ring overhead outweighs freeing the Q7 prep — keep at 96)

NCORES = 8
SELU_L = 1.0507009873554804934193349852946
SELU_A = 1.6732632423543772848170429916717
SA = SELU_L * SELU_A
NOMATCH = 999.0

DEFAULT_CFG = dict(N=100000, E=3200000, IN_DIM=256, HID=64, NCLS=32, EPS=1e-5,
                   Q=4, CH=6144, GB=12)


def _derive(cfg):
    d = dict(cfg)
    N, Q = d["N"], d["Q"]
    R = N // NCORES
    assert R * NCORES == N and R % Q == 0
    RQ = R // Q
    W = -(-R // 128)
    d.update(R=R, RQ=RQ, W=W, R_pad=W * 128, SLAB=NCORES * RQ)
    assert d["SLAB"] < 32768, "slab must fit int16 indices"
    assert d["IN_DIM"] % 128 == 0
    return d


def _build_schedule(src, dst, cfg):
    """Uniform cross-core edge schedule.

    Returns per-core srcidx/dstloc arrays and the compile-time meta
    (groups-per-(bucket,window) Mg, bucket lengths L, window of each group).
    """
    N, Q, R, RQ, W = cfg["N"], cfg["Q"], cfg["R"], cfg["RQ"], cfg["W"]
    src = np.asarray(src).astype(np.int64).ravel()
    dst = np.asarray(dst).astype(np.int64).ravel()

    c_e = dst // R
    w_e = (dst % R) // 128
    dloc = (dst % R) % 128
    b_e = (src % R) // RQ
    sloc = (src // R) * RQ + (src % R) % RQ          # row within slab b_e

    key = (c_e * Q + b_e) * W + w_e
    order = np.argsort(key, kind="stable")
    cnt = np.bincount(key, minlength=NCORES * Q * W).reshape(NCORES, Q, W)

    Mg = -(-cnt.max(axis=0) // 128)                  # [Q, W] groups per (b,w)
    seg_len = Mg * 128
    # slot offset of segment (b, w) inside bucket b's stream
    seg_off = np.zeros((Q, W), np.int64)
    L = np.zeros(Q, np.int64)
    for b in range(Q):
        seg_off[b] = np.concatenate([[0], np.cumsum(seg_len[b])[:-1]])
        L[b] = seg_len[b].sum()

    # per-edge slot position: seg_off[b,w] + rank within its (c,b,w) run
    key_s = key[order]
    run_start = np.r_[0, np.flatnonzero(np.diff(key_s)) + 1]
    run_id = np.zeros(len(key_s), np.int64)
    run_id[run_start[1:]] = 1
    run_id = np.cumsum(run_id)
    rank = np.arange(len(key_s)) - run_start[run_id]
    slot = seg_off[b_e[order], w_e[order]] + rank

    srcidx = [[np.zeros(L[b], np.int16) for b in range(Q)] for _ in range(NCORES)]
    dstloc = [[np.full(L[b], NOMATCH, np.float32) for b in range(Q)] for _ in range(NCORES)]
    ce_s, be_s = c_e[order], b_e[order]
    sl_s, dl_s = sloc[order], dloc[order]
    for c in range(NCORES):
        mc = ce_s == c
        for b in range(Q):
            m = mc & (be_s == b)
            srcidx[c][b][slot[m]] = sl_s[m].astype(np.int16)
            dstloc[c][b][slot[m]] = dl_s[m].astype(np.float32)

    win_of_group = [np.repeat(np.arange(W), Mg[b]) for b in range(Q)]
    return srcidx, dstloc, dict(Mg=Mg, L=L, win_of_group=win_of_group)


def _wrap16(a):
    """[L] -> [128, L/16] int16, 16-wrapped and replicated to 128 partitions."""
    L = a.shape[0]
    w = a.reshape(L // 16, 16).T
    return np.ascontiguousarray(np.tile(w, (8, 1)))


def _slotmajor(a):
    """[L] -> [128, L/128]: slot j at [j%128, j//128]."""
    L = a.shape[0]
    return np.ascontiguousarray(a.reshape(L // 128, 128).T)


def _rowtile(v, cfg):
    """[R] (+pad zeros) -> [128, W]: row r at [r%128, r//128]."""
    W = cfg["W"]
    out = np.zeros((128, W), np.float32)
    full = np.zeros(W * 128, np.float32)
    full[: v.shape[0]] = v
    return np.ascontiguousarray(full.reshape(W, 128).T)


def _build_nc(cfg, meta):
    N, Q, W, CH, GB = cfg["N"], cfg["Q"], cfg["W"], cfg["CH"], cfg["GB"]
    R, RQ, R_pad, SLAB = cfg["R"], cfg["RQ"], cfg["R_pad"], cfg["SLAB"]
    IN_DIM, HID, NCLS, EPS = cfg["IN_DIM"], cfg["HID"], cfg["NCLS"], cfg["EPS"]
    KC = IN_DIM // 128
    L = meta["L"]
    wog = meta["win_of_group"]
    last_rows = R - (W - 1) * 128

    nc = bacc.Bacc("TRN2", target_bir_lowering=False, debug=False,
                   num_devices=NCORES, num_swdge_queues=4)

    # ---- I/O ----
    xT_ext = nc.dram_tensor("xT", [IN_DIM, R_pad], F32, kind="ExternalInput")
    w1_ext = nc.dram_tensor("w1", [IN_DIM, HID], F32, kind="ExternalInput")
    w2_ext = nc.dram_tensor("w2", [HID, NCLS], F32, kind="ExternalInput")
    b1r_ext = nc.dram_tensor("b1r", [128, HID], F32, kind="ExternalInput")
    b2r_ext = nc.dram_tensor("b2r", [128, NCLS], F32, kind="ExternalInput")
    gb_ext = nc.dram_tensor("gb", [1, 2 * HID], F32, kind="ExternalInput")
    dinv_ext = nc.dram_tensor("dinv_t", [128, W], F32, kind="ExternalInput")
    wself_ext = nc.dram_tensor("wself_t", [128, W], F32, kind="ExternalInput")
    sidx_ext = [nc.dram_tensor(f"sidx{b}", [128, int(L[b]) // 16], I16,
                               kind="ExternalInput") for b in range(Q)]
    dloc_ext = [nc.dram_tensor(f"dloc{b}", [128, int(L[b]) // 128], BF16,
                               kind="ExternalInput") for b in range(Q)]
    out_ext = nc.dram_tensor("out", [R, NCLS], F32, kind="ExternalOutput")

    # ---- internals ----
    u_local = nc.dram_tensor("u_local", [R_pad, HID], F32)
    slabs = [nc.dram_tensor(f"slab{b}", [SLAB, HID], F32, addr_space="Shared")
             for b in range(Q)]
    bn_in = nc.dram_tensor("bn_in", [1, 2 * HID], F32)
    bn_out = nc.dram_tensor("bn_out", [1, 2 * HID], F32, addr_space="Shared")

    # iota_full[p, i, j] = i  (j innermost so the one-hot build reads both
    # inputs at stride 1; the matmul reads oh2[:, :, j] as strided lhsT)
    iota_np = np.ascontiguousarray(np.broadcast_to(
        np.arange(128, dtype=np.float32)[None, :, None],
        (128, 128, GB))).astype(_BF16)
    ident_np = np.eye(128, dtype=np.float32)
    onescol_np = np.ones((1, 128), np.float32)
    mask_np = np.zeros((128, 2), np.float32)
    mask_np[:, 0] = 1.0
    mask_np[:last_rows, 1] = 1.0
    iota_d = nc.inline_tensor(iota_np, "iota_c")
    ident_d = nc.inline_tensor(ident_np, "ident_c")
    onescol_d = nc.inline_tensor(onescol_np, "onescol_c")
    mask_d = nc.inline_tensor(mask_np, "mask_c")

    rg = [list(range(NCORES))]
    AG = "AllGather"

    from contextlib import ExitStack
    with tile.TileContext(nc) as tc, ExitStack() as est:
        cpool = est.enter_context(tc.tile_pool(name="consts", bufs=1))
        ppool = est.enter_context(tc.tile_pool(name="persist", bufs=1))
        xpool = est.enter_context(tc.tile_pool(name="xin", bufs=3))
        mpool = est.enter_context(tc.tile_pool(name="msg", bufs=4))
        bpool = est.enter_context(tc.tile_pool(name="msgb", bufs=3))
        ipool = est.enter_context(tc.tile_pool(name="idx", bufs=4))
        dpool = est.enter_context(tc.tile_pool(name="dloc", bufs=4))
        opool = est.enter_context(tc.tile_pool(name="oh", bufs=4))
        qpool = est.enter_context(tc.tile_pool(name="qtl", bufs=3))
        ps_mm = est.enter_context(tc.tile_pool(name="psmm", bufs=4, space="PSUM"))
        ps_aux = est.enter_context(tc.tile_pool(name="psaux", bufs=1, space="PSUM"))

        # constants to SBUF
        iota_s = cpool.tile([128, 128, GB], BF16, tag="iota")
        ident_s = cpool.tile([128, 128], F32, tag="ident")
        onescol_s = cpool.tile([1, 128], F32, tag="onescol")
        mask_s = cpool.tile([128, 2], F32, tag="mask")
        w1_s = cpool.tile([128, KC, HID], F32, tag="w1")
        w2_s = cpool.tile([HID, NCLS], F32, tag="w2")
        b1r_s = cpool.tile([128, HID], F32, tag="b1r")
        b2r_s = cpool.tile([128, NCLS], F32, tag="b2r")
        gb_s = cpool.tile([1, 2 * HID], F32, tag="gb")
        dinv_s = cpool.tile([128, W], F32, tag="dinv")
        wself_s = cpool.tile([128, W], F32, tag="wself")
        nc.sync.dma_start(out=iota_s[:], in_=iota_d[:])
        nc.sync.dma_start(out=ident_s[:], in_=ident_d[:])
        nc.sync.dma_start(out=onescol_s[:], in_=onescol_d[:])
        nc.sync.dma_start(out=mask_s[:], in_=mask_d[:])
        for k in range(KC):
            nc.sync.dma_start(out=w1_s[:, k, :], in_=w1_ext[k * 128:(k + 1) * 128, :])
        nc.sync.dma_start(out=w2_s[:], in_=w2_ext[:])
        nc.sync.dma_start(out=b1r_s[:], in_=b1r_ext[:])
        nc.sync.dma_start(out=b2r_s[:], in_=b2r_ext[:])
        nc.sync.dma_start(out=gb_s[:], in_=gb_ext[:])
        nc.sync.dma_start(out=dinv_s[:], in_=dinv_ext[:])
        nc.sync.dma_start(out=wself_s[:], in_=wself_ext[:])

        u_own = ppool.tile([128, W, HID], F32, tag="u_own")
        acc = ppool.tile([128, W, HID], F32, tag="acc")
        scr = ppool.tile([128, W, HID], F32, tag="scr")
        z_st = ppool.tile([128, W, NCLS], F32, tag="z_st")
        sume = ppool.tile([128, W], F32, tag="sume")

        # ---------- phase 1: u0 = dinv * (x @ W1) ----------
        w1b = cpool.tile([128, KC, HID], BF16, tag="w1b")
        nc.scalar.copy(w1b[:], w1_s[:])
        for t in range(W):
            xt = xpool.tile([128, KC, 128], F32, tag="xt")
            nc.sync.dma_start(
                out=xt[:],
                in_=xT_ext[:, t * 128:(t + 1) * 128].rearrange(
                    "(k p) f -> p k f", k=KC))
            xb = xpool.tile([128, KC, 128], BF16, tag="xb")
            nc.scalar.copy(xb[:], xt[:])
            py = ps_mm.tile([128, HID], F32, tag="mm")
            for k in range(KC):
                nc.tensor.matmul(py[:], lhsT=xb[:, k, :], rhs=w1b[:, k, :],
                                 start=(k == 0), stop=(k == KC - 1))
            nc.vector.tensor_scalar_mul(u_own[:, t, :], py[:], dinv_s[:, t:t + 1])
        nc.sync.dma_start(
            out=u_local[:].rearrange("(t p) f -> p t f", p=128), in_=u_own[:])

        # ---------- propagation ----------
        qctr = [0]

        # first bucket touching each window (its drain writes, later ones add)
        first_b = {}
        for b in range(Q):
            for w_ in sorted(set(int(x) for x in meta["win_of_group"][b])):
                first_b.setdefault(w_, b)
        assert sorted(first_b) == list(range(W)), "window never touched"

        def prop_step(step):
            for b in range(Q):
                nc.gpsimd.collective_compute(
                    AG, mybir.AluOpType.bypass, replica_groups=rg,
                    ins=[u_local[b * RQ:b * RQ + RQ, :]], outs=[slabs[b][:]])
            for b in range(Q):
                Lb = int(L[b])
                ngroups = Lb // 128
                cur_psum = None
                cur_w = -1
                chunks = []
                c0 = 0
                while c0 < Lb:
                    cl = min(CH, Lb - c0)
                    chunks.append((c0, cl))
                    c0 += cl
                for (c0, cl) in chunks:
                    gn_c = cl // 128
                    it = ipool.tile([128, CH // 16], I16, tag="it")
                    nc.sync.dma_start(
                        out=it[:, : cl // 16],
                        in_=sidx_ext[b][:, c0 // 16:(c0 + cl) // 16])
                    dl = dpool.tile([128, CH // 128], BF16, tag="dl")
                    nc.sync.dma_start(
                        out=dl[:, : cl // 128],
                        in_=dloc_ext[b][:, c0 // 128:(c0 + cl) // 128])
                    mt = mpool.tile([128, CH // 128, HID], F32, tag="mt")
                    # q0 preps block the Pool SEQ for the full desc-gen
                    # (cpu 0 is both ack-master and queue-0 worker); q1-3
                    # are fire-and-forget.  Issue q0 LAST in each round so
                    # the three background pairs are already working.
                    nc.gpsimd.dma_gather(
                        mt[:, : gn_c, :], slabs[b][:], it[:, : cl // 16],
                        cl, cl, HID, single_packet=False,
                        queue_num=(1, 2, 3, 0)[qctr[0] % 4])
                    qctr[0] += 1
                    mtb = bpool.tile([128, CH // 128, HID], BF16, tag="mtb")
                    nc.scalar.copy(mtb[:, : gn_c, :], mt[:, : gn_c, :])
                    g0 = c0 // 128
                    for blk in range(0, gn_c, GB):
                        bw = min(GB, gn_c - blk)
                        # oh[p, i, j] = (dloc[p, blk+j] == i); built full-GB
                        # wide so every AP is innermost-stride-1 (junk
                        # columns past bw are never consumed)
                        oh = opool.tile([128, 128, GB], BF16, tag="oh")
                        nc.vector.tensor_tensor(
                            oh[:],
                            dl[:, blk:blk + GB].unsqueeze(1).to_broadcast(
                                [128, 128, GB]),
                            iota_s[:],
                            mybir.AluOpType.is_equal)
                        for j in range(bw):
                            g = g0 + blk + j
                            w = int(wog[b][g])
                            if w != cur_w:
                                cur_psum = ps_mm.tile([128, HID], F32, tag="mm")
                                cur_w = w
                                first = True
                            else:
                                first = False
                            last = (g == ngroups - 1) or int(wog[b][g + 1]) != w
                            nc.tensor.matmul(
                                cur_psum[:], lhsT=oh[:, :, j],
                                rhs=mtb[:, blk + j, :],
                                start=first, stop=last)
                            if last:
                                if first_b[w] == b:
                                    nc.vector.tensor_copy(
                                        acc[:, w, :], cur_psum[:])
                                else:
                                    nc.vector.tensor_add(
                                        acc[:, w, :], acc[:, w, :], cur_psum[:])

        def store_u():
            nc.sync.dma_start(
                out=u_local[:].rearrange("(t p) f -> p t f", p=128),
                in_=u_own[:])

        # ---- step 0: u1 = wself * ((A+I) u0) ----
        prop_step(0)
        nc.vector.tensor_add(u_own[:], u_own[:], acc[:])
        for t in range(W):
            nc.vector.tensor_scalar_mul(u_own[:, t, :], u_own[:, t, :],
                                        wself_s[:, t:t + 1])
        store_u()

        # ---- step 1: h = dinv * ((A+I) u1) + b1; BN; selu; u2 = dinv*hb ----
        prop_step(1)
        nc.vector.tensor_add(u_own[:], u_own[:], acc[:])
        for t in range(W):
            nc.vector.tensor_scalar_mul(u_own[:, t, :], u_own[:, t, :],
                                        dinv_s[:, t:t + 1])
        nc.vector.tensor_add(
            u_own[:], u_own[:],
            b1r_s[:].unsqueeze(1).to_broadcast([128, W, HID]))
        # BN stats: sum h and sum h^2 over valid rows
        ps_s1 = ps_aux.tile([1, HID], F32, tag="s1")
        ps_s2 = ps_aux.tile([1, HID], F32, tag="s2")
        nc.vector.tensor_mul(scr[:], u_own[:], u_own[:])
        for t in range(W):
            mcol = mask_s[:, 1:2] if t == W - 1 else mask_s[:, 0:1]
            nc.tensor.matmul(ps_s1[:], lhsT=mcol, rhs=u_own[:, t, :],
                             start=(t == 0), stop=(t == W - 1))
            nc.tensor.matmul(ps_s2[:], lhsT=mcol, rhs=scr[:, t, :],
                             start=(t == 0), stop=(t == W - 1))
        stat_s = cpool.tile([1, 2 * HID], F32, tag="stat")
        nc.scalar.copy(stat_s[:, :HID], ps_s1[:])
        nc.scalar.copy(stat_s[:, HID:], ps_s2[:])
        nc.sync.dma_start(out=bn_in[:], in_=stat_s[:])
        nc.gpsimd.collective_compute(
            "AllReduce", mybir.AluOpType.add, replica_groups=rg,
            ins=[bn_in[:]], outs=[bn_out[:]])
        bnst = cpool.tile([1, 2 * HID], F32, tag="bnst")
        nc.sync.dma_start(out=bnst[:], in_=bn_out[:])
        mean_s = cpool.tile([1, HID], F32, tag="mean")
        var_s = cpool.tile([1, HID], F32, tag="var")
        coef_s = cpool.tile([1, 2 * HID], F32, tag="coef")
        nc.vector.tensor_scalar_mul(mean_s[:], bnst[:, :HID], 1.0 / N)
        nc.vector.tensor_scalar_mul(var_s[:], bnst[:, HID:], 1.0 / N)
        msq = cpool.tile([1, HID], F32, tag="msq")
        nc.vector.tensor_mul(msq[:], mean_s[:], mean_s[:])
        nc.vector.tensor_sub(var_s[:], var_s[:], msq[:])
        nc.vector.tensor_scalar_add(var_s[:], var_s[:], float(EPS))
        nc.vector.reciprocal(var_s[:], var_s[:])
        nc.scalar.sqrt(var_s[:], var_s[:])          # var_s = 1/sqrt(var+eps)
        # coef = [scale, shift]; scale = gamma*inv, shift = beta - mean*scale
        nc.vector.tensor_mul(coef_s[:, :HID], gb_s[:, :HID], var_s[:])
        nc.vector.tensor_mul(msq[:], mean_s[:], coef_s[:, :HID])
        nc.vector.tensor_sub(coef_s[:, HID:], gb_s[:, HID:], msq[:])
        ps_bc = ps_aux.tile([128, 2 * HID], F32, tag="bc")
        nc.tensor.matmul(ps_bc[:], lhsT=onescol_s[:], rhs=coef_s[:],
                         start=True, stop=True)
        coefr = cpool.tile([128, 2 * HID], F32, tag="coefr")
        nc.vector.tensor_copy(coefr[:], ps_bc[:])
        # hb = h*scale + shift ; selu(hb) ; u2 = dinv*selu
        nc.vector.tensor_mul(
            u_own[:], u_own[:],
            coefr[:, :HID].unsqueeze(1).to_broadcast([128, W, HID]))
        nc.vector.tensor_add(
            u_own[:], u_own[:],
            coefr[:, HID:].unsqueeze(1).to_broadcast([128, W, HID]))
        nc.vector.tensor_scalar_min(scr[:], u_own[:], 0.0)
        nc.scalar.activation(scr[:], scr[:],
                             mybir.ActivationFunctionType.Exp)
        # u_own = SELU_L*relu(h) + SA*exp(min(h,0)) - SA
        nc.vector.tensor_scalar_max(u_own[:], u_own[:], 0.0)
        nc.vector.tensor_scalar(u_own[:], u_own[:],
                                SELU_L, None, mybir.AluOpType.mult)
        nc.vector.tensor_scalar(scr[:], scr[:], SA, -SA,
                                mybir.AluOpType.mult, mybir.AluOpType.add)
        nc.vector.tensor_add(u_own[:], u_own[:], scr[:])
        for t in range(W):
            nc.vector.tensor_scalar_mul(u_own[:, t, :], u_own[:, t, :],
                                        dinv_s[:, t:t + 1])
        store_u()

        # ---- step 2: q = dinv*((A+I) u2); z = q@W2 + b2; log_softmax ----
        prop_step(2)
        for t in range(W):
            qt = qpool.tile([128, HID], F32, tag="qt")
            nc.vector.tensor_add(qt[:], u_own[:, t, :], acc[:, t, :])
            nc.vector.tensor_scalar_mul(qt[:], qt[:], dinv_s[:, t:t + 1])
            ps_qT = ps_mm.tile([HID, 128], F32, tag="mm")
            nc.tensor.transpose(out=ps_qT[:], in_=qt[:], identity=ident_s[:])
            qT = qpool.tile([HID, 128], F32, tag="qTs")
            nc.vector.tensor_copy(qT[:], ps_qT[:])
            ps_z = ps_mm.tile([128, NCLS], F32, tag="mm")
            nc.tensor.matmul(ps_z[:], lhsT=qT[:], rhs=w2_s[:],
                             start=True, stop=True)
            nc.vector.tensor_add(z_st[:, t, :], ps_z[:], b2r_s[:])
        rmax = cpool.tile([128, 1], F32, tag="rmax")
        for t in range(W):
            nc.vector.tensor_reduce(rmax[:], z_st[:, t, :],
                                    mybir.AxisListType.X, mybir.AluOpType.max)
            nc.vector.tensor_scalar(z_st[:, t, :], z_st[:, t, :], rmax[:],
                                    None, mybir.AluOpType.subtract)
        for t in range(W):
            nc.scalar.activation(scr[:, t, :NCLS], z_st[:, t, :],
                                 mybir.ActivationFunctionType.Exp,
                                 accum_out=sume[:, t:t + 1])
        nc.scalar.activation(sume[:], sume[:], mybir.ActivationFunctionType.Ln)
        nc.vector.tensor_scalar_mul(sume[:], sume[:], -1.0)
        for t in range(W):
            nc.vector.tensor_scalar(z_st[:, t, :], z_st[:, t, :],
                                    sume[:, t:t + 1], None,
                                    mybir.AluOpType.add)
            rows = last_rows if t == W - 1 else 128
            nc.sync.dma_start(out=out_ext[t * 128:t * 128 + rows, :],
                              in_=z_st[:rows, t, :])

    return nc


def _prepare_inputs(inputs, cfg, sched):
    """Per-core in_maps from full inputs."""
    srcidx, dstloc, meta = sched
    N, Q, R, W = cfg["N"], cfg["Q"], cfg["R"], cfg["W"]
    R_pad = cfg["R_pad"]
    x = np.asarray(inputs["x"], np.float32)
    src = np.asarray(inputs["src"]).astype(np.int64).ravel()
    dst = np.asarray(inputs["dst"]).astype(np.int64).ravel()
    W1 = np.asarray(inputs["W1"], np.float32)
    b1 = np.asarray(inputs["b1"], np.float32)
    gamma = np.asarray(inputs["gamma"], np.float32)
    beta = np.asarray(inputs["beta"], np.float32)
    W2 = np.asarray(inputs["W2"], np.float32)
    b2 = np.asarray(inputs["b2"], np.float32)

    deg = np.bincount(dst, minlength=N).astype(np.float32) + 1.0
    dinv = 1.0 / np.sqrt(deg)
    wself = 1.0 / deg

    gb = np.concatenate([gamma, beta])[None, :]
    b1r = np.tile(b1[None, :], (128, 1))
    b2r = np.tile(b2[None, :], (128, 1))

    in_maps = []
    for c in range(NCORES):
        xc = x[c * R:(c + 1) * R]
        xT = np.zeros((cfg["IN_DIM"], R_pad), np.float32)
        xT[:, :R] = xc.T
        m = {
            "xT": np.ascontiguousarray(xT),
            "w1": W1, "w2": W2, "b1r": b1r, "b2r": b2r, "gb": gb,
            "dinv_t": _rowtile(dinv[c * R:(c + 1) * R], cfg),
            "wself_t": _rowtile(wself[c * R:(c + 1) * R], cfg),
        }
        for b in range(Q):
            m[f"sidx{b}"] = _wrap16(srcidx[c][b])
            m[f"dloc{b}"] = _slotmajor(dstloc[c][b]).astype(_BF16)
        in_maps.append(m)
    return in_maps


def build_all(inputs, cfg=None):
    cfg = _derive(cfg or DEFAULT_CFG)
    sched = _build_schedule(inputs["src"], inputs["dst"], cfg)
    nc = _build_nc(cfg, sched[2])
    in_maps = _prepare_inputs(inputs, cfg, sched)
    return nc, in_maps, cfg


def kernel(**inputs):
    import concourse.bass_utils as _bu
    _bu.upload_artifacts = lambda tmpdir: f"file://{tmpdir}"  # offline container
    nc, in_maps, cfg = build_all(inputs)
    nc.compile()
    res = run_bass_kernel_spmd(nc, in_maps, list(range(NCORES)))
    out = np.concatenate([res.results[c]["out"] for c in range(NCORES)], axis=0)
    return out.astype(np.float32)



# revision 30
# speedup vs baseline: 1.0440x; 1.0440x over previous
"""GCN node-classifier forward on 8 Trainium2 NeuronCores.

out = log_softmax(P(selu(BN(P^2 (x W1) + b1))) W2 + b2),
P = D^-1/2 (A+I) D^-1/2.

Strategy: nodes row-sharded across 8 cores; edges partitioned by dst core and
sorted by (src-quarter, dst-window).  Per propagation step: the per-node scaled
features u = dinv*h are AllGathered into 4 quarter "slabs" (quarter-permuted
row order so each AllGather produces one contiguous slab); each core gathers
u[src] for its edges via dma_gather (int16 indices are valid within one slab)
and segment-sums them into its dst windows with one-hot matmuls accumulating
in PSUM.  The uniform (max over cores) padded schedule keeps the SPMD
instruction stream identical on every core.
"""

import sys

sys.path.insert(0, "/opt/trn_rl_repo")

import numpy as np
from ml_dtypes import bfloat16 as _BF16

import concourse.bacc as bacc
import concourse.tile as tile
from concourse import bass, mybir
from concourse.bass_utils import run_bass_kernel_spmd

F32 = mybir.dt.float32
BF16 = mybir.dt.bfloat16
I16 = mybir.dt.int16
I32 = mybir.dt.int32

G_Q7 = 96   # groups per 96-group chunk gathered via Q7 swdge dma_gather;
            # any remainder would go through the hardware indirect-DMA
            # walker (measured slower per 128-row call: its per-instruction
            # ring overhead outweighs freeing the Q7 prep — keep at 96)

NCORES = 8
SELU_L = 1.0507009873554804934193349852946
SELU_A = 1.6732632423543772848170429916717
SA = SELU_L * SELU_A
NOMATCH = 999.0

DEFAULT_CFG = dict(N=100000, E=3200000, IN_DIM=256, HID=64, NCLS=32, EPS=1e-5,
                   Q=4, CH=6144, GB=12)


def _derive(cfg):
    d = dict(cfg)
    N, Q = d["N"], d["Q"]
    R = N // NCORES
    assert R * NCORES == N and R % Q == 0
    RQ = R // Q
    W = -(-R // 128)
    d.update(R=R, RQ=RQ, W=W, R_pad=W * 128, SLAB=NCORES * RQ)
    assert d["SLAB"] < 32768, "slab must fit int16 indices"
    assert d["IN_DIM"] % 128 == 0
    return d


def _build_schedule(src, dst, cfg):
    """Uniform cross-core edge schedule.

    Returns per-core srcidx/dstloc arrays and the compile-time meta
    (groups-per-(bucket,window) Mg, bucket lengths L, window of each group).
    """
    N, Q, R, RQ, W = cfg["N"], cfg["Q"], cfg["R"], cfg["RQ"], cfg["W"]
    src = np.asarray(src).astype(np.int64).ravel()
    dst = np.asarray(dst).astype(np.int64).ravel()

    c_e = dst // R
    w_e = (dst % R) // 128
    dloc = (dst % R) % 128
    b_e = (src % R) // RQ
    sloc = (src // R) * RQ + (src % R) % RQ          # row within slab b_e

    key = (c_e * Q + b_e) * W + w_e
    order = np.argsort(key, kind="stable")
    cnt = np.bincount(key, minlength=NCORES * Q * W).reshape(NCORES, Q, W)

    Mg = -(-cnt.max(axis=0) // 128)                  # [Q, W] groups per (b,w)
    seg_len = Mg * 128
    # slot offset of segment (b, w) inside bucket b's stream
    seg_off = np.zeros((Q, W), np.int64)
    L = np.zeros(Q, np.int64)
    for b in range(Q):
        seg_off[b] = np.concatenate([[0], np.cumsum(seg_len[b])[:-1]])
        L[b] = seg_len[b].sum()

    # per-edge slot position: seg_off[b,w] + rank within its (c,b,w) run
    key_s = key[order]
    run_start = np.r_[0, np.flatnonzero(np.diff(key_s)) + 1]
    run_id = np.zeros(len(key_s), np.int64)
    run_id[run_start[1:]] = 1
    run_id = np.cumsum(run_id)
    rank = np.arange(len(key_s)) - run_start[run_id]
    slot = seg_off[b_e[order], w_e[order]] + rank

    srcidx = [[np.zeros(L[b], np.int16) for b in range(Q)] for _ in range(NCORES)]
    dstloc = [[np.full(L[b], NOMATCH, np.float32) for b in range(Q)] for _ in range(NCORES)]
    ce_s, be_s = c_e[order], b_e[order]
    sl_s, dl_s = sloc[order], dloc[order]
    for c in range(NCORES):
        mc = ce_s == c
        for b in range(Q):
            m = mc & (be_s == b)
            srcidx[c][b][slot[m]] = sl_s[m].astype(np.int16)
            dstloc[c][b][slot[m]] = dl_s[m].astype(np.float32)

    win_of_group = [np.repeat(np.arange(W), Mg[b]) for b in range(Q)]
    return srcidx, dstloc, dict(Mg=Mg, L=L, win_of_group=win_of_group)


def _wrap16(a):
    """[L] -> [128, L/16] int16, 16-wrapped and replicated to 128 partitions."""
    L = a.shape[0]
    w = a.reshape(L // 16, 16).T
    return np.ascontiguousarray(np.tile(w, (8, 1)))


def _slotmajor(a):
    """[L] -> [128, L/128]: slot j at [j%128, j//128]."""
    L = a.shape[0]
    return np.ascontiguousarray(a.reshape(L // 128, 128).T)


def _rowtile(v, cfg):
    """[R] (+pad zeros) -> [128, W]: row r at [r%128, r//128]."""
    W = cfg["W"]
    out = np.zeros((128, W), np.float32)
    full = np.zeros(W * 128, np.float32)
    full[: v.shape[0]] = v
    return np.ascontiguousarray(full.reshape(W, 128).T)


def _build_nc(cfg, meta):
    N, Q, W, CH, GB = cfg["N"], cfg["Q"], cfg["W"], cfg["CH"], cfg["GB"]
    R, RQ, R_pad, SLAB = cfg["R"], cfg["RQ"], cfg["R_pad"], cfg["SLAB"]
    IN_DIM, HID, NCLS, EPS = cfg["IN_DIM"], cfg["HID"], cfg["NCLS"], cfg["EPS"]
    KC = IN_DIM // 128
    L = meta["L"]
    wog = meta["win_of_group"]
    last_rows = R - (W - 1) * 128

    nc = bacc.Bacc("TRN2", target_bir_lowering=False, debug=False,
                   num_devices=NCORES, num_swdge_queues=4)

    # ---- I/O ----
    xT_ext = nc.dram_tensor("xT", [IN_DIM, R_pad], F32, kind="ExternalInput")
    w1_ext = nc.dram_tensor("w1", [IN_DIM, HID], F32, kind="ExternalInput")
    w2_ext = nc.dram_tensor("w2", [HID, NCLS], F32, kind="ExternalInput")
    b1r_ext = nc.dram_tensor("b1r", [128, HID], F32, kind="ExternalInput")
    b2r_ext = nc.dram_tensor("b2r", [128, NCLS], F32, kind="ExternalInput")
    gb_ext = nc.dram_tensor("gb", [1, 2 * HID], F32, kind="ExternalInput")
    dinv_ext = nc.dram_tensor("dinv_t", [128, W], F32, kind="ExternalInput")
    wself_ext = nc.dram_tensor("wself_t", [128, W], F32, kind="ExternalInput")
    sidx_ext = [nc.dram_tensor(f"sidx{b}", [128, int(L[b]) // 16], I16,
                               kind="ExternalInput") for b in range(Q)]
    dloc_ext = [nc.dram_tensor(f"dloc{b}", [128, int(L[b]) // 128], BF16,
                               kind="ExternalInput") for b in range(Q)]
    out_ext = nc.dram_tensor("out", [R, NCLS], F32, kind="ExternalOutput")

    # ---- internals ----
    u_local = nc.dram_tensor("u_local", [R_pad, HID], F32)
    slabs = [nc.dram_tensor(f"slab{b}", [SLAB, HID], F32, addr_space="Shared")
             for b in range(Q)]
    bn_in = nc.dram_tensor("bn_in", [1, 2 * HID], F32)
    bn_out = nc.dram_tensor("bn_out", [1, 2 * HID], F32, addr_space="Shared")

    # iota_full[p, i, j] = i  (j innermost so the one-hot build reads both
    # inputs at stride 1; the matmul reads oh2[:, :, j] as strided lhsT)
    iota_np = np.ascontiguousarray(np.broadcast_to(
        np.arange(128, dtype=np.float32)[None, :, None],
        (128, 128, GB))).astype(_BF16)
    ident_np = np.eye(128, dtype=np.float32)
    onescol_np = np.ones((1, 128), np.float32)
    mask_np = np.zeros((128, 2), np.float32)
    mask_np[:, 0] = 1.0
    mask_np[:last_rows, 1] = 1.0
    iota_d = nc.inline_tensor(iota_np, "iota_c")
    ident_d = nc.inline_tensor(ident_np, "ident_c")
    onescol_d = nc.inline_tensor(onescol_np, "onescol_c")
    mask_d = nc.inline_tensor(mask_np, "mask_c")

    rg = [list(range(NCORES))]
    AG = "AllGather"

    from contextlib import ExitStack
    with tile.TileContext(nc) as tc, ExitStack() as est:
        cpool = est.enter_context(tc.tile_pool(name="consts", bufs=1))
        ppool = est.enter_context(tc.tile_pool(name="persist", bufs=1))
        xpool = est.enter_context(tc.tile_pool(name="xin", bufs=3))
        mpool = est.enter_context(tc.tile_pool(name="msg", bufs=5))
        bpool = est.enter_context(tc.tile_pool(name="msgb", bufs=4))
        ipool = est.enter_context(tc.tile_pool(name="idx", bufs=6))
        dpool = est.enter_context(tc.tile_pool(name="dloc", bufs=6))
        opool = est.enter_context(tc.tile_pool(name="oh", bufs=4))
        qpool = est.enter_context(tc.tile_pool(name="qtl", bufs=3))
        ps_mm = est.enter_context(tc.tile_pool(name="psmm", bufs=4, space="PSUM"))
        ps_aux = est.enter_context(tc.tile_pool(name="psaux", bufs=1, space="PSUM"))

        # constants to SBUF
        iota_s = cpool.tile([128, 128, GB], BF16, tag="iota")
        ident_s = cpool.tile([128, 128], F32, tag="ident")
        onescol_s = cpool.tile([1, 128], F32, tag="onescol")
        mask_s = cpool.tile([128, 2], F32, tag="mask")
        w1_s = cpool.tile([128, KC, HID], F32, tag="w1")
        w2_s = cpool.tile([HID, NCLS], F32, tag="w2")
        b1r_s = cpool.tile([128, HID], F32, tag="b1r")
        b2r_s = cpool.tile([128, NCLS], F32, tag="b2r")
        gb_s = cpool.tile([1, 2 * HID], F32, tag="gb")
        dinv_s = cpool.tile([128, W], F32, tag="dinv")
        wself_s = cpool.tile([128, W], F32, tag="wself")
        nc.sync.dma_start(out=iota_s[:], in_=iota_d[:])
        nc.sync.dma_start(out=ident_s[:], in_=ident_d[:])
        nc.sync.dma_start(out=onescol_s[:], in_=onescol_d[:])
        nc.sync.dma_start(out=mask_s[:], in_=mask_d[:])
        for k in range(KC):
            nc.sync.dma_start(out=w1_s[:, k, :], in_=w1_ext[k * 128:(k + 1) * 128, :])
        nc.sync.dma_start(out=w2_s[:], in_=w2_ext[:])
        nc.sync.dma_start(out=b1r_s[:], in_=b1r_ext[:])
        nc.sync.dma_start(out=b2r_s[:], in_=b2r_ext[:])
        nc.sync.dma_start(out=gb_s[:], in_=gb_ext[:])
        nc.sync.dma_start(out=dinv_s[:], in_=dinv_ext[:])
        nc.sync.dma_start(out=wself_s[:], in_=wself_ext[:])

        u_own = ppool.tile([128, W, HID], F32, tag="u_own")
        acc = ppool.tile([128, W, HID], F32, tag="acc")
        scr = ppool.tile([128, W, HID], F32, tag="scr")
        z_st = ppool.tile([128, W, NCLS], F32, tag="z_st")
        sume = ppool.tile([128, W], F32, tag="sume")

        # ---------- phase 1: u0 = dinv * (x @ W1) ----------
        w1b = cpool.tile([128, KC, HID], BF16, tag="w1b")
        nc.scalar.copy(w1b[:], w1_s[:])
        for t in range(W):
            xt = xpool.tile([128, KC, 128], F32, tag="xt")
            nc.sync.dma_start(
                out=xt[:],
                in_=xT_ext[:, t * 128:(t + 1) * 128].rearrange(
                    "(k p) f -> p k f", k=KC))
            xb = xpool.tile([128, KC, 128], BF16, tag="xb")
            nc.scalar.copy(xb[:], xt[:])
            py = ps_mm.tile([128, HID], F32, tag="mm")
            for k in range(KC):
                nc.tensor.matmul(py[:], lhsT=xb[:, k, :], rhs=w1b[:, k, :],
                                 start=(k == 0), stop=(k == KC - 1))
            nc.vector.tensor_scalar_mul(u_own[:, t, :], py[:], dinv_s[:, t:t + 1])
        nc.sync.dma_start(
            out=u_local[:].rearrange("(t p) f -> p t f", p=128), in_=u_own[:])

        # ---------- propagation ----------
        qctr = [0]

        # first bucket touching each window (its drain writes, later ones add)
        first_b = {}
        for b in range(Q):
            for w_ in sorted(set(int(x) for x in meta["win_of_group"][b])):
                first_b.setdefault(w_, b)
        assert sorted(first_b) == list(range(W)), "window never touched"

        def prop_step(step):
            for b in range(Q):
                nc.gpsimd.collective_compute(
                    AG, mybir.AluOpType.bypass, replica_groups=rg,
                    ins=[u_local[b * RQ:b * RQ + RQ, :]], outs=[slabs[b][:]])
            for b in range(Q):
                Lb = int(L[b])
                ngroups = Lb // 128
                cur_psum = None
                cur_w = -1
                chunks = []
                c0 = 0
                while c0 < Lb:
                    cl = min(CH, Lb - c0)
                    chunks.append((c0, cl))
                    c0 += cl
                for (c0, cl) in chunks:
                    gn_c = cl // 128
                    it = ipool.tile([128, CH // 16], I16, tag="it")
                    nc.sync.dma_start(
                        out=it[:, : cl // 16],
                        in_=sidx_ext[b][:, c0 // 16:(c0 + cl) // 16])
                    dl = dpool.tile([128, CH // 128], BF16, tag="dl")
                    nc.sync.dma_start(
                        out=dl[:, : cl // 128],
                        in_=dloc_ext[b][:, c0 // 128:(c0 + cl) // 128])
                    mt = mpool.tile([128, CH // 128, HID], F32, tag="mt")
                    # q0 preps block the Pool SEQ for the full desc-gen
                    # (cpu 0 is both ack-master and queue-0 worker); q1-3
                    # are fire-and-forget.  Issue q0 LAST in each round so
                    # the three background pairs are already working.
                    nc.gpsimd.dma_gather(
                        mt[:, : gn_c, :], slabs[b][:], it[:, : cl // 16],
                        cl, cl, HID, single_packet=False,
                        queue_num=(1, 2, 3, 0)[qctr[0] % 4])
                    qctr[0] += 1
                    mtb = bpool.tile([128, CH // 128, HID], BF16, tag="mtb")
                    nc.scalar.copy(mtb[:, : gn_c, :], mt[:, : gn_c, :])
                    g0 = c0 // 128
                    for blk in range(0, gn_c, GB):
                        bw = min(GB, gn_c - blk)
                        # oh[p, i, j] = (dloc[p, blk+j] == i); built full-GB
                        # wide so every AP is innermost-stride-1 (junk
                        # columns past bw are never consumed)
                        oh = opool.tile([128, 128, GB], BF16, tag="oh")
                        nc.vector.tensor_tensor(
                            oh[:],
                            dl[:, blk:blk + GB].unsqueeze(1).to_broadcast(
                                [128, 128, GB]),
                            iota_s[:],
                            mybir.AluOpType.is_equal)
                        for j in range(bw):
                            g = g0 + blk + j
                            w = int(wog[b][g])
                            if w != cur_w:
                                cur_psum = ps_mm.tile([128, HID], F32, tag="mm")
                                cur_w = w
                                first = True
                            else:
                                first = False
                            last = (g == ngroups - 1) or int(wog[b][g + 1]) != w
                            nc.tensor.matmul(
                                cur_psum[:], lhsT=oh[:, :, j],
                                rhs=mtb[:, blk + j, :],
                                start=first, stop=last)
                            if last:
                                if first_b[w] == b:
                                    nc.vector.tensor_copy(
                                        acc[:, w, :], cur_psum[:])
                                else:
                                    nc.vector.tensor_add(
                                        acc[:, w, :], acc[:, w, :], cur_psum[:])

        def store_u():
            nc.sync.dma_start(
                out=u_local[:].rearrange("(t p) f -> p t f", p=128),
                in_=u_own[:])

        # ---- step 0: u1 = wself * ((A+I) u0) ----
        prop_step(0)
        nc.vector.tensor_add(u_own[:], u_own[:], acc[:])
        for t in range(W):
            nc.vector.tensor_scalar_mul(u_own[:, t, :], u_own[:, t, :],
                                        wself_s[:, t:t + 1])
        store_u()

        # ---- step 1: h = dinv * ((A+I) u1) + b1; BN; selu; u2 = dinv*hb ----
        prop_step(1)
        nc.vector.tensor_add(u_own[:], u_own[:], acc[:])
        for t in range(W):
            nc.vector.tensor_scalar_mul(u_own[:, t, :], u_own[:, t, :],
                                        dinv_s[:, t:t + 1])
        nc.vector.tensor_add(
            u_own[:], u_own[:],
            b1r_s[:].unsqueeze(1).to_broadcast([128, W, HID]))
        # BN stats: sum h and sum h^2 over valid rows
        ps_s1 = ps_aux.tile([1, HID], F32, tag="s1")
        ps_s2 = ps_aux.tile([1, HID], F32, tag="s2")
        nc.vector.tensor_mul(scr[:], u_own[:], u_own[:])
        for t in range(W):
            mcol = mask_s[:, 1:2] if t == W - 1 else mask_s[:, 0:1]
            nc.tensor.matmul(ps_s1[:], lhsT=mcol, rhs=u_own[:, t, :],
                             start=(t == 0), stop=(t == W - 1))
            nc.tensor.matmul(ps_s2[:], lhsT=mcol, rhs=scr[:, t, :],
                             start=(t == 0), stop=(t == W - 1))
        stat_s = cpool.tile([1, 2 * HID], F32, tag="stat")
        nc.scalar.copy(stat_s[:, :HID], ps_s1[:])
        nc.scalar.copy(stat_s[:, HID:], ps_s2[:])
        nc.sync.dma_start(out=bn_in[:], in_=stat_s[:])
        nc.gpsimd.collective_compute(
            "AllReduce", mybir.AluOpType.add, replica_groups=rg,
            ins=[bn_in[:]], outs=[bn_out[:]])
        bnst = cpool.tile([1, 2 * HID], F32, tag="bnst")
        nc.sync.dma_start(out=bnst[:], in_=bn_out[:])
        mean_s = cpool.tile([1, HID], F32, tag="mean")
        var_s = cpool.tile([1, HID], F32, tag="var")
        coef_s = cpool.tile([1, 2 * HID], F32, tag="coef")
        nc.vector.tensor_scalar_mul(mean_s[:], bnst[:, :HID], 1.0 / N)
        nc.vector.tensor_scalar_mul(var_s[:], bnst[:, HID:], 1.0 / N)
        msq = cpool.tile([1, HID], F32, tag="msq")
        nc.vector.tensor_mul(msq[:], mean_s[:], mean_s[:])
        nc.vector.tensor_sub(var_s[:], var_s[:], msq[:])
        nc.vector.tensor_scalar_add(var_s[:], var_s[:], float(EPS))
        nc.vector.reciprocal(var_s[:], var_s[:])
        nc.scalar.sqrt(var_s[:], var_s[:])          # var_s = 1/sqrt(var+eps)
        # coef = [scale, shift]; scale = gamma*inv, shift = beta - mean*scale
        nc.vector.tensor_mul(coef_s[:, :HID], gb_s[:, :HID], var_s[:])
        nc.vector.tensor_mul(msq[:], mean_s[:], coef_s[:, :HID])
        nc.vector.tensor_sub(coef_s[:, HID:], gb_s[:, HID:], msq[:])
        ps_bc = ps_aux.tile([128, 2 * HID], F32, tag="bc")
        nc.tensor.matmul(ps_bc[:], lhsT=onescol_s[:], rhs=coef_s[:],
                         start=True, stop=True)
        coefr = cpool.tile([128, 2 * HID], F32, tag="coefr")
        nc.vector.tensor_copy(coefr[:], ps_bc[:])
        # hb = h*scale + shift ; selu(hb) ; u2 = dinv*selu
        nc.vector.tensor_mul(
            u_own[:], u_own[:],
            coefr[:, :HID].unsqueeze(1).to_broadcast([128, W, HID]))
        nc.vector.tensor_add(
            u_own[:], u_own[:],
            coefr[:, HID:].unsqueeze(1).to_broadcast([128, W, HID]))
        nc.vector.tensor_scalar_min(scr[:], u_own[:], 0.0)
        nc.scalar.activation(scr[:], scr[:],
                             mybir.ActivationFunctionType.Exp)
        # u_own = SELU_L*relu(h) + SA*exp(min(h,0)) - SA
        nc.vector.tensor_scalar_max(u_own[:], u_own[:], 0.0)
        nc.vector.tensor_scalar(u_own[:], u_own[:],
                                SELU_L, None, mybir.AluOpType.mult)
        nc.vector.tensor_scalar(scr[:], scr[:], SA, -SA,
                                mybir.AluOpType.mult, mybir.AluOpType.add)
        nc.vector.tensor_add(u_own[:], u_own[:], scr[:])
        for t in range(W):
            nc.vector.tensor_scalar_mul(u_own[:, t, :], u_own[:, t, :],
                                        dinv_s[:, t:t + 1])
        store_u()

        # ---- step 2: q = dinv*((A+I) u2); z = q@W2 + b2; log_softmax ----
        prop_step(2)
        for t in range(W):
            qt = qpool.tile([128, HID], F32, tag="qt")
            nc.vector.tensor_add(qt[:], u_own[:, t, :], acc[:, t, :])
            nc.vector.tensor_scalar_mul(qt[:], qt[:], dinv_s[:, t:t + 1])
            ps_qT = ps_mm.tile([HID, 128], F32, tag="mm")
            nc.tensor.transpose(out=ps_qT[:], in_=qt[:], identity=ident_s[:])
            qT = qpool.tile([HID, 128], F32, tag="qTs")
            nc.vector.tensor_copy(qT[:], ps_qT[:])
            ps_z = ps_mm.tile([128, NCLS], F32, tag="mm")
            nc.tensor.matmul(ps_z[:], lhsT=qT[:], rhs=w2_s[:],
                             start=True, stop=True)
            nc.vector.tensor_add(z_st[:, t, :], ps_z[:], b2r_s[:])
        rmax = cpool.tile([128, 1], F32, tag="rmax")
        for t in range(W):
            nc.vector.tensor_reduce(rmax[:], z_st[:, t, :],
                                    mybir.AxisListType.X, mybir.AluOpType.max)
            nc.vector.tensor_scalar(z_st[:, t, :], z_st[:, t, :], rmax[:],
                                    None, mybir.AluOpType.subtract)
        for t in range(W):
            nc.scalar.activation(scr[:, t, :NCLS], z_st[:, t, :],
                                 mybir.ActivationFunctionType.Exp,
                                 accum_out=sume[:, t:t + 1])
        nc.scalar.activation(sume[:], sume[:], mybir.ActivationFunctionType.Ln)
        nc.vector.tensor_scalar_mul(sume[:], sume[:], -1.0)
        for t in range(W):
            nc.vector.tensor_scalar(z_st[:, t, :], z_st[:, t, :],
                                    sume[:, t:t + 1], None,
                                    mybir.AluOpType.add)
            rows = last_rows if t == W - 1 else 128
            nc.sync.dma_start(out=out_ext[t * 128:t * 128 + rows, :],
                              in_=z_st[:rows, t, :])

    return nc


def _prepare_inputs(inputs, cfg, sched):
    """Per-core in_maps from full inputs."""
    srcidx, dstloc, meta = sched
    N, Q, R, W = cfg["N"], cfg["Q"], cfg["R"], cfg["W"]
    R_pad = cfg["R_pad"]
    x = np.asarray(inputs["x"], np.float32)
    src = np.asarray(inputs["src"]).astype(np.int64).ravel()
    dst = np.asarray(inputs["dst"]).astype(np.int64).ravel()
    W1 = np.asarray(inputs["W1"], np.float32)
    b1 = np.asarray(inputs["b1"], np.float32)
    gamma = np.asarray(inputs["gamma"], np.float32)
    beta = np.asarray(inputs["beta"], np.float32)
    W2 = np.asarray(inputs["W2"], np.float32)
    b2 = np.asarray(inputs["b2"], np.float32)

    deg = np.bincount(dst, minlength=N).astype(np.float32) + 1.0
    dinv = 1.0 / np.sqrt(deg)
    wself = 1.0 / deg

    gb = np.concatenate([gamma, beta])[None, :]
    b1r = np.tile(b1[None, :], (128, 1))
    b2r = np.tile(b2[None, :], (128, 1))

    in_maps = []
    for c in range(NCORES):
        xc = x[c * R:(c + 1) * R]
        xT = np.zeros((cfg["IN_DIM"], R_pad), np.float32)
        xT[:, :R] = xc.T
        m = {
            "xT": np.ascontiguousarray(xT),
            "w1": W1, "w2": W2, "b1r": b1r, "b2r": b2r, "gb": gb,
            "dinv_t": _rowtile(dinv[c * R:(c + 1) * R], cfg),
            "wself_t": _rowtile(wself[c * R:(c + 1) * R], cfg),
        }
        for b in range(Q):
            m[f"sidx{b}"] = _wrap16(srcidx[c][b])
            m[f"dloc{b}"] = _slotmajor(dstloc[c][b]).astype(_BF16)
        in_maps.append(m)
    return in_maps


def build_all(inputs, cfg=None):
    cfg = _derive(cfg or DEFAULT_CFG)
    sched = _build_schedule(inputs["src"], inputs["dst"], cfg)
    nc = _build_nc(cfg, sched[2])
    in_maps = _prepare_inputs(inputs, cfg, sched)
    return nc, in_maps, cfg


def kernel(**inputs):
    import concourse.bass_utils as _bu
    _bu.upload_artifacts = lambda tmpdir: f"file://{tmpdir}"  # offline container
    nc, in_maps, cfg = build_all(inputs)
    nc.compile()
    res = run_bass_kernel_spmd(nc, in_maps, list(range(NCORES)))
    out = np.concatenate([res.results[c]["out"] for c in range(NCORES)], axis=0)
    return out.astype(np.float32)



# revision 34
# speedup vs baseline: 1.0988x; 1.0525x over previous
"""GCN node-classifier forward on 8 Trainium2 NeuronCores.

out = log_softmax(P(selu(BN(P^2 (x W1) + b1))) W2 + b2),
P = D^-1/2 (A+I) D^-1/2.

Strategy: nodes row-sharded across 8 cores; edges partitioned by dst core and
sorted by (src-quarter, dst-window).  Per propagation step: the per-node scaled
features u = dinv*h are AllGathered into 4 quarter "slabs" (quarter-permuted
row order so each AllGather produces one contiguous slab); each core gathers
u[src] for its edges via dma_gather (int16 indices are valid within one slab)
and segment-sums them into its dst windows with one-hot matmuls accumulating
in PSUM.  The uniform (max over cores) padded schedule keeps the SPMD
instruction stream identical on every core.
"""

import sys

sys.path.insert(0, "/opt/trn_rl_repo")

import numpy as np
from ml_dtypes import bfloat16 as _BF16

import concourse.bacc as bacc
import concourse.tile as tile
from concourse import bass, mybir
from concourse.bass_utils import run_bass_kernel_spmd

F32 = mybir.dt.float32
BF16 = mybir.dt.bfloat16
I16 = mybir.dt.int16
I32 = mybir.dt.int32

G_Q7 = 96   # groups per 96-group chunk gathered via Q7 swdge dma_gather;
            # any remainder would go through the hardware indirect-DMA
            # walker (measured slower per 128-row call: its per-instruction
            # ring overhead outweighs freeing the Q7 prep — keep at 96)

NCORES = 8
SELU_L = 1.0507009873554804934193349852946
SELU_A = 1.6732632423543772848170429916717
SA = SELU_L * SELU_A
NOMATCH = 999.0

DEFAULT_CFG = dict(N=100000, E=3200000, IN_DIM=256, HID=64, NCLS=32, EPS=1e-5,
                   Q=4, CH=6144, GB=12)


def _derive(cfg):
    d = dict(cfg)
    N, Q = d["N"], d["Q"]
    R = N // NCORES
    assert R * NCORES == N and R % Q == 0
    RQ = R // Q
    W = -(-R // 128)
    d.update(R=R, RQ=RQ, W=W, R_pad=W * 128, SLAB=NCORES * RQ)
    assert d["SLAB"] < 32768, "slab must fit int16 indices"
    assert d["IN_DIM"] % 128 == 0
    return d


def _build_schedule(src, dst, cfg):
    """Uniform cross-core edge schedule.

    Returns per-core srcidx/dstloc arrays and the compile-time meta
    (groups-per-(bucket,window) Mg, bucket lengths L, window of each group).
    """
    N, Q, R, RQ, W = cfg["N"], cfg["Q"], cfg["R"], cfg["RQ"], cfg["W"]
    src = np.asarray(src).astype(np.int64).ravel()
    dst = np.asarray(dst).astype(np.int64).ravel()

    c_e = dst // R
    w_e = (dst % R) // 128
    dloc = (dst % R) % 128
    b_e = (src % R) // RQ
    sloc = (src // R) * RQ + (src % R) % RQ          # row within slab b_e

    key = (c_e * Q + b_e) * W + w_e
    order = np.argsort(key, kind="stable")
    cnt = np.bincount(key, minlength=NCORES * Q * W).reshape(NCORES, Q, W)

    Mg = -(-cnt.max(axis=0) // 128)                  # [Q, W] groups per (b,w)
    seg_len = Mg * 128
    # slot offset of segment (b, w) inside bucket b's stream
    seg_off = np.zeros((Q, W), np.int64)
    L = np.zeros(Q, np.int64)
    for b in range(Q):
        seg_off[b] = np.concatenate([[0], np.cumsum(seg_len[b])[:-1]])
        L[b] = seg_len[b].sum()

    # per-edge slot position: seg_off[b,w] + rank within its (c,b,w) run
    key_s = key[order]
    run_start = np.r_[0, np.flatnonzero(np.diff(key_s)) + 1]
    run_id = np.zeros(len(key_s), np.int64)
    run_id[run_start[1:]] = 1
    run_id = np.cumsum(run_id)
    rank = np.arange(len(key_s)) - run_start[run_id]
    slot = seg_off[b_e[order], w_e[order]] + rank

    srcidx = [[np.zeros(L[b], np.int16) for b in range(Q)] for _ in range(NCORES)]
    dstloc = [[np.full(L[b], NOMATCH, np.float32) for b in range(Q)] for _ in range(NCORES)]
    ce_s, be_s = c_e[order], b_e[order]
    sl_s, dl_s = sloc[order], dloc[order]
    for c in range(NCORES):
        mc = ce_s == c
        for b in range(Q):
            m = mc & (be_s == b)
            srcidx[c][b][slot[m]] = sl_s[m].astype(np.int16)
            dstloc[c][b][slot[m]] = dl_s[m].astype(np.float32)

    win_of_group = [np.repeat(np.arange(W), Mg[b]) for b in range(Q)]
    return srcidx, dstloc, dict(Mg=Mg, L=L, win_of_group=win_of_group)


def _wrap16(a):
    """[L] -> [128, L/16] int16, 16-wrapped and replicated to 128 partitions."""
    L = a.shape[0]
    w = a.reshape(L // 16, 16).T
    return np.ascontiguousarray(np.tile(w, (8, 1)))


def _slotmajor(a):
    """[L] -> [128, L/128]: slot j at [j%128, j//128]."""
    L = a.shape[0]
    return np.ascontiguousarray(a.reshape(L // 128, 128).T)


def _rowtile(v, cfg):
    """[R] (+pad zeros) -> [128, W]: row r at [r%128, r//128]."""
    W = cfg["W"]
    out = np.zeros((128, W), np.float32)
    full = np.zeros(W * 128, np.float32)
    full[: v.shape[0]] = v
    return np.ascontiguousarray(full.reshape(W, 128).T)


def _build_nc(cfg, meta):
    N, Q, W, CH, GB = cfg["N"], cfg["Q"], cfg["W"], cfg["CH"], cfg["GB"]
    R, RQ, R_pad, SLAB = cfg["R"], cfg["RQ"], cfg["R_pad"], cfg["SLAB"]
    IN_DIM, HID, NCLS, EPS = cfg["IN_DIM"], cfg["HID"], cfg["NCLS"], cfg["EPS"]
    KC = IN_DIM // 128
    L = meta["L"]
    wog = meta["win_of_group"]
    last_rows = R - (W - 1) * 128

    nc = bacc.Bacc("TRN2", target_bir_lowering=False, debug=False,
                   num_devices=NCORES, num_swdge_queues=4)

    # ---- I/O ----
    xT_ext = nc.dram_tensor("xT", [IN_DIM, R_pad], F32, kind="ExternalInput")
    w1_ext = nc.dram_tensor("w1", [IN_DIM, HID], F32, kind="ExternalInput")
    w2_ext = nc.dram_tensor("w2", [HID, NCLS], F32, kind="ExternalInput")
    b1r_ext = nc.dram_tensor("b1r", [128, HID], F32, kind="ExternalInput")
    b2r_ext = nc.dram_tensor("b2r", [128, NCLS], F32, kind="ExternalInput")
    gb_ext = nc.dram_tensor("gb", [1, 2 * HID], F32, kind="ExternalInput")
    dinv_ext = nc.dram_tensor("dinv_t", [128, W], F32, kind="ExternalInput")
    wself_ext = nc.dram_tensor("wself_t", [128, W], F32, kind="ExternalInput")
    sidx_ext = [nc.dram_tensor(f"sidx{b}", [128, int(L[b]) // 16], I16,
                               kind="ExternalInput") for b in range(Q)]
    dloc_ext = [nc.dram_tensor(f"dloc{b}", [128, int(L[b]) // 128], BF16,
                               kind="ExternalInput") for b in range(Q)]
    out_ext = nc.dram_tensor("out", [R, NCLS], F32, kind="ExternalOutput")

    # ---- internals ----
    u_local = nc.dram_tensor("u_local", [R_pad, HID], F32)
    slabs = [nc.dram_tensor(f"slab{b}", [SLAB, HID], F32, addr_space="Shared")
             for b in range(Q)]
    bn_in = nc.dram_tensor("bn_in", [1, 2 * HID], F32)
    bn_out = nc.dram_tensor("bn_out", [1, 2 * HID], F32, addr_space="Shared")

    # iota_full[p, i, j] = i  (j innermost so the one-hot build reads both
    # inputs at stride 1; the matmul reads oh2[:, :, j] as strided lhsT)
    iota_np = np.ascontiguousarray(np.broadcast_to(
        np.arange(128, dtype=np.float32)[None, :, None],
        (128, 128, GB))).astype(_BF16)
    ident_np = np.eye(128, dtype=np.float32)
    onescol_np = np.ones((1, 128), np.float32)
    mask_np = np.zeros((128, 2), np.float32)
    mask_np[:, 0] = 1.0
    mask_np[:last_rows, 1] = 1.0
    iota_d = nc.inline_tensor(iota_np, "iota_c")
    ident_d = nc.inline_tensor(ident_np, "ident_c")
    onescol_d = nc.inline_tensor(onescol_np, "onescol_c")
    mask_d = nc.inline_tensor(mask_np, "mask_c")

    rg = [list(range(NCORES))]
    AG = "AllGather"

    from contextlib import ExitStack
    with tile.TileContext(nc) as tc, ExitStack() as est:
        cpool = est.enter_context(tc.tile_pool(name="consts", bufs=1))
        ppool = est.enter_context(tc.tile_pool(name="persist", bufs=1))
        xpool = est.enter_context(tc.tile_pool(name="xin", bufs=3))
        mpool = est.enter_context(tc.tile_pool(name="msg", bufs=5))
        bpool = est.enter_context(tc.tile_pool(name="msgb", bufs=4))
        ipool = est.enter_context(tc.tile_pool(name="idx", bufs=6))
        dpool = est.enter_context(tc.tile_pool(name="dloc", bufs=2))
        opool = est.enter_context(tc.tile_pool(name="oh", bufs=4))
        qpool = est.enter_context(tc.tile_pool(name="qtl", bufs=3))
        ps_mm = est.enter_context(tc.tile_pool(name="psmm", bufs=4, space="PSUM"))
        ps_aux = est.enter_context(tc.tile_pool(name="psaux", bufs=1, space="PSUM"))

        # constants to SBUF
        iota_s = cpool.tile([128, 128, GB], BF16, tag="iota")
        ident_s = cpool.tile([128, 128], F32, tag="ident")
        onescol_s = cpool.tile([1, 128], F32, tag="onescol")
        mask_s = cpool.tile([128, 2], F32, tag="mask")
        w1_s = cpool.tile([128, KC, HID], F32, tag="w1")
        w2_s = cpool.tile([HID, NCLS], F32, tag="w2")
        b1r_s = cpool.tile([128, HID], F32, tag="b1r")
        b2r_s = cpool.tile([128, NCLS], F32, tag="b2r")
        gb_s = cpool.tile([1, 2 * HID], F32, tag="gb")
        dinv_s = cpool.tile([128, W], F32, tag="dinv")
        wself_s = cpool.tile([128, W], F32, tag="wself")
        nc.sync.dma_start(out=iota_s[:], in_=iota_d[:])
        nc.sync.dma_start(out=ident_s[:], in_=ident_d[:])
        nc.sync.dma_start(out=onescol_s[:], in_=onescol_d[:])
        nc.sync.dma_start(out=mask_s[:], in_=mask_d[:])
        for k in range(KC):
            nc.sync.dma_start(out=w1_s[:, k, :], in_=w1_ext[k * 128:(k + 1) * 128, :])
        nc.sync.dma_start(out=w2_s[:], in_=w2_ext[:])
        nc.sync.dma_start(out=b1r_s[:], in_=b1r_ext[:])
        nc.sync.dma_start(out=b2r_s[:], in_=b2r_ext[:])
        nc.sync.dma_start(out=gb_s[:], in_=gb_ext[:])
        nc.sync.dma_start(out=dinv_s[:], in_=dinv_ext[:])
        nc.sync.dma_start(out=wself_s[:], in_=wself_ext[:])

        u_own = ppool.tile([128, W, HID], F32, tag="u_own")
        acc = ppool.tile([128, W, HID], F32, tag="acc")
        scr = ppool.tile([128, W, HID], F32, tag="scr")
        z_st = ppool.tile([128, W, NCLS], F32, tag="z_st")
        sume = ppool.tile([128, W], F32, tag="sume")

        # ---------- phase 1: u0 = dinv * (x @ W1) ----------
        w1b = cpool.tile([128, KC, HID], BF16, tag="w1b")
        nc.scalar.copy(w1b[:], w1_s[:])
        for t in range(W):
            xt = xpool.tile([128, KC, 128], F32, tag="xt")
            nc.sync.dma_start(
                out=xt[:],
                in_=xT_ext[:, t * 128:(t + 1) * 128].rearrange(
                    "(k p) f -> p k f", k=KC))
            xb = xpool.tile([128, KC, 128], BF16, tag="xb")
            nc.scalar.copy(xb[:], xt[:])
            py = ps_mm.tile([128, HID], F32, tag="mm")
            for k in range(KC):
                nc.tensor.matmul(py[:], lhsT=xb[:, k, :], rhs=w1b[:, k, :],
                                 start=(k == 0), stop=(k == KC - 1))
            nc.vector.tensor_scalar_mul(u_own[:, t, :], py[:], dinv_s[:, t:t + 1])
        nc.sync.dma_start(
            out=u_local[:].rearrange("(t p) f -> p t f", p=128), in_=u_own[:])

        # ---------- propagation ----------
        qctr = [0]

        # first bucket touching each window (its drain writes, later ones add)
        first_b = {}
        for b in range(Q):
            for w_ in sorted(set(int(x) for x in meta["win_of_group"][b])):
                first_b.setdefault(w_, b)
        assert sorted(first_b) == list(range(W)), "window never touched"

        def prop_step(step):
            for b in range(Q):
                nc.gpsimd.collective_compute(
                    AG, mybir.AluOpType.bypass, replica_groups=rg,
                    ins=[u_local[b * RQ:b * RQ + RQ, :]], outs=[slabs[b][:]])
            for b in range(Q):
                Lb = int(L[b])
                ngroups = Lb // 128
                cur_psum = None
                cur_w = -1
                # whole bucket's dloc in one DMA (tiny per-chunk loads are
                # descriptor-dominated and compete with the gather drain)
                dlb = dpool.tile([128, int(max(L)) // 128 + GB], BF16,
                                 tag="dlb")
                nc.sync.dma_start(out=dlb[:, : Lb // 128], in_=dloc_ext[b][:])
                chunks = []
                c0 = 0
                while c0 < Lb:
                    cl = min(CH, Lb - c0)
                    chunks.append((c0, cl))
                    c0 += cl
                for (c0, cl) in chunks:
                    gn_c = cl // 128
                    it = ipool.tile([128, CH // 16], I16, tag="it")
                    nc.sync.dma_start(
                        out=it[:, : cl // 16],
                        in_=sidx_ext[b][:, c0 // 16:(c0 + cl) // 16])
                    mt = mpool.tile([128, CH // 128, HID], F32, tag="mt")
                    # q0 preps block the Pool SEQ for the full desc-gen
                    # (cpu 0 is both ack-master and queue-0 worker); q1-3
                    # are fire-and-forget.  Issue q0 LAST in each round so
                    # the three background pairs are already working.
                    nc.gpsimd.dma_gather(
                        mt[:, : gn_c, :], slabs[b][:], it[:, : cl // 16],
                        cl, cl, HID, single_packet=False,
                        queue_num=(1, 2, 3, 0)[qctr[0] % 4])
                    qctr[0] += 1
                    mtb = bpool.tile([128, CH // 128, HID], BF16, tag="mtb")
                    nc.scalar.copy(mtb[:, : gn_c, :], mt[:, : gn_c, :])
                    g0 = c0 // 128
                    for blk in range(0, gn_c, GB):
                        bw = min(GB, gn_c - blk)
                        # oh[p, i, j] = (dloc[p, blk+j] == i); built full-GB
                        # wide so every AP is innermost-stride-1 (junk
                        # columns past bw are never consumed)
                        oh = opool.tile([128, 128, GB], BF16, tag="oh")
                        nc.vector.tensor_tensor(
                            oh[:],
                            dlb[:, g0 + blk:g0 + blk + GB].unsqueeze(
                                1).to_broadcast([128, 128, GB]),
                            iota_s[:],
                            mybir.AluOpType.is_equal)
                        for j in range(bw):
                            g = g0 + blk + j
                            w = int(wog[b][g])
                            if w != cur_w:
                                cur_psum = ps_mm.tile([128, HID], F32, tag="mm")
                                cur_w = w
                                first = True
                            else:
                                first = False
                            last = (g == ngroups - 1) or int(wog[b][g + 1]) != w
                            nc.tensor.matmul(
                                cur_psum[:], lhsT=oh[:, :, j],
                                rhs=mtb[:, blk + j, :],
                                start=first, stop=last)
                            if last:
                                if first_b[w] == b:
                                    nc.vector.tensor_copy(
                                        acc[:, w, :], cur_psum[:])
                                else:
                                    nc.vector.tensor_add(
                                        acc[:, w, :], acc[:, w, :], cur_psum[:])

        def store_u():
            nc.sync.dma_start(
                out=u_local[:].rearrange("(t p) f -> p t f", p=128),
                in_=u_own[:])

        # ---- step 0: u1 = wself * ((A+I) u0) ----
        prop_step(0)
        nc.vector.tensor_add(u_own[:], u_own[:], acc[:])
        for t in range(W):
            nc.vector.tensor_scalar_mul(u_own[:, t, :], u_own[:, t, :],
                                        wself_s[:, t:t + 1])
        store_u()

        # ---- step 1: h = dinv * ((A+I) u1) + b1; BN; selu; u2 = dinv*hb ----
        prop_step(1)
        nc.vector.tensor_add(u_own[:], u_own[:], acc[:])
        for t in range(W):
            nc.vector.tensor_scalar_mul(u_own[:, t, :], u_own[:, t, :],
                                        dinv_s[:, t:t + 1])
        nc.vector.tensor_add(
            u_own[:], u_own[:],
            b1r_s[:].unsqueeze(1).to_broadcast([128, W, HID]))
        # BN stats: sum h and sum h^2 over valid rows
        ps_s1 = ps_aux.tile([1, HID], F32, tag="s1")
        ps_s2 = ps_aux.tile([1, HID], F32, tag="s2")
        nc.vector.tensor_mul(scr[:], u_own[:], u_own[:])
        for t in range(W):
            mcol = mask_s[:, 1:2] if t == W - 1 else mask_s[:, 0:1]
            nc.tensor.matmul(ps_s1[:], lhsT=mcol, rhs=u_own[:, t, :],
                             start=(t == 0), stop=(t == W - 1))
            nc.tensor.matmul(ps_s2[:], lhsT=mcol, rhs=scr[:, t, :],
                             start=(t == 0), stop=(t == W - 1))
        stat_s = cpool.tile([1, 2 * HID], F32, tag="stat")
        nc.scalar.copy(stat_s[:, :HID], ps_s1[:])
        nc.scalar.copy(stat_s[:, HID:], ps_s2[:])
        nc.sync.dma_start(out=bn_in[:], in_=stat_s[:])
        nc.gpsimd.collective_compute(
            "AllReduce", mybir.AluOpType.add, replica_groups=rg,
            ins=[bn_in[:]], outs=[bn_out[:]])
        bnst = cpool.tile([1, 2 * HID], F32, tag="bnst")
        nc.sync.dma_start(out=bnst[:], in_=bn_out[:])
        mean_s = cpool.tile([1, HID], F32, tag="mean")
        var_s = cpool.tile([1, HID], F32, tag="var")
        coef_s = cpool.tile([1, 2 * HID], F32, tag="coef")
        nc.vector.tensor_scalar_mul(mean_s[:], bnst[:, :HID], 1.0 / N)
        nc.vector.tensor_scalar_mul(var_s[:], bnst[:, HID:], 1.0 / N)
        msq = cpool.tile([1, HID], F32, tag="msq")
        nc.vector.tensor_mul(msq[:], mean_s[:], mean_s[:])
        nc.vector.tensor_sub(var_s[:], var_s[:], msq[:])
        nc.vector.tensor_scalar_add(var_s[:], var_s[:], float(EPS))
        nc.vector.reciprocal(var_s[:], var_s[:])
        nc.scalar.sqrt(var_s[:], var_s[:])          # var_s = 1/sqrt(var+eps)
        # coef = [scale, shift]; scale = gamma*inv, shift = beta - mean*scale
        nc.vector.tensor_mul(coef_s[:, :HID], gb_s[:, :HID], var_s[:])
        nc.vector.tensor_mul(msq[:], mean_s[:], coef_s[:, :HID])
        nc.vector.tensor_sub(coef_s[:, HID:], gb_s[:, HID:], msq[:])
        ps_bc = ps_aux.tile([128, 2 * HID], F32, tag="bc")
        nc.tensor.matmul(ps_bc[:], lhsT=onescol_s[:], rhs=coef_s[:],
                         start=True, stop=True)
        coefr = cpool.tile([128, 2 * HID], F32, tag="coefr")
        nc.vector.tensor_copy(coefr[:], ps_bc[:])
        # hb = h*scale + shift ; selu(hb) ; u2 = dinv*selu
        nc.vector.tensor_mul(
            u_own[:], u_own[:],
            coefr[:, :HID].unsqueeze(1).to_broadcast([128, W, HID]))
        nc.vector.tensor_add(
            u_own[:], u_own[:],
            coefr[:, HID:].unsqueeze(1).to_broadcast([128, W, HID]))
        nc.vector.tensor_scalar_min(scr[:], u_own[:], 0.0)
        nc.scalar.activation(scr[:], scr[:],
                             mybir.ActivationFunctionType.Exp)
        # u_own = SELU_L*relu(h) + SA*exp(min(h,0)) - SA
        nc.vector.tensor_scalar_max(u_own[:], u_own[:], 0.0)
        nc.vector.tensor_scalar(u_own[:], u_own[:],
                                SELU_L, None, mybir.AluOpType.mult)
        nc.vector.tensor_scalar(scr[:], scr[:], SA, -SA,
                                mybir.AluOpType.mult, mybir.AluOpType.add)
        nc.vector.tensor_add(u_own[:], u_own[:], scr[:])
        for t in range(W):
            nc.vector.tensor_scalar_mul(u_own[:, t, :], u_own[:, t, :],
                                        dinv_s[:, t:t + 1])
        store_u()

        # ---- step 2: q = dinv*((A+I) u2); z = q@W2 + b2; log_softmax ----
        prop_step(2)
        for t in range(W):
            qt = qpool.tile([128, HID], F32, tag="qt")
            nc.vector.tensor_add(qt[:], u_own[:, t, :], acc[:, t, :])
            nc.vector.tensor_scalar_mul(qt[:], qt[:], dinv_s[:, t:t + 1])
            ps_qT = ps_mm.tile([HID, 128], F32, tag="mm")
            nc.tensor.transpose(out=ps_qT[:], in_=qt[:], identity=ident_s[:])
            qT = qpool.tile([HID, 128], F32, tag="qTs")
            nc.vector.tensor_copy(qT[:], ps_qT[:])
            ps_z = ps_mm.tile([128, NCLS], F32, tag="mm")
            nc.tensor.matmul(ps_z[:], lhsT=qT[:], rhs=w2_s[:],
                             start=True, stop=True)
            nc.vector.tensor_add(z_st[:, t, :], ps_z[:], b2r_s[:])
        rmax = cpool.tile([128, 1], F32, tag="rmax")
        for t in range(W):
            nc.vector.tensor_reduce(rmax[:], z_st[:, t, :],
                                    mybir.AxisListType.X, mybir.AluOpType.max)
            nc.vector.tensor_scalar(z_st[:, t, :], z_st[:, t, :], rmax[:],
                                    None, mybir.AluOpType.subtract)
        for t in range(W):
            nc.scalar.activation(scr[:, t, :NCLS], z_st[:, t, :],
                                 mybir.ActivationFunctionType.Exp,
                                 accum_out=sume[:, t:t + 1])
        nc.scalar.activation(sume[:], sume[:], mybir.ActivationFunctionType.Ln)
        nc.vector.tensor_scalar_mul(sume[:], sume[:], -1.0)
        for t in range(W):
            nc.vector.tensor_scalar(z_st[:, t, :], z_st[:, t, :],
                                    sume[:, t:t + 1], None,
                                    mybir.AluOpType.add)
            rows = last_rows if t == W - 1 else 128
            nc.sync.dma_start(out=out_ext[t * 128:t * 128 + rows, :],
                              in_=z_st[:rows, t, :])

    return nc


def _prepare_inputs(inputs, cfg, sched):
    """Per-core in_maps from full inputs."""
    srcidx, dstloc, meta = sched
    N, Q, R, W = cfg["N"], cfg["Q"], cfg["R"], cfg["W"]
    R_pad = cfg["R_pad"]
    x = np.asarray(inputs["x"], np.float32)
    src = np.asarray(inputs["src"]).astype(np.int64).ravel()
    dst = np.asarray(inputs["dst"]).astype(np.int64).ravel()
    W1 = np.asarray(inputs["W1"], np.float32)
    b1 = np.asarray(inputs["b1"], np.float32)
    gamma = np.asarray(inputs["gamma"], np.float32)
    beta = np.asarray(inputs["beta"], np.float32)
    W2 = np.asarray(inputs["W2"], np.float32)
    b2 = np.asarray(inputs["b2"], np.float32)

    deg = np.bincount(dst, minlength=N).astype(np.float32) + 1.0
    dinv = 1.0 / np.sqrt(deg)
    wself = 1.0 / deg

    gb = np.concatenate([gamma, beta])[None, :]
    b1r = np.tile(b1[None, :], (128, 1))
    b2r = np.tile(b2[None, :], (128, 1))

    in_maps = []
    for c in range(NCORES):
        xc = x[c * R:(c + 1) * R]
        xT = np.zeros((cfg["IN_DIM"], R_pad), np.float32)
        xT[:, :R] = xc.T
        m = {
            "xT": np.ascontiguousarray(xT),
            "w1": W1, "w2": W2, "b1r": b1r, "b2r": b2r, "gb": gb,
            "dinv_t": _rowtile(dinv[c * R:(c + 1) * R], cfg),
            "wself_t": _rowtile(wself[c * R:(c + 1) * R], cfg),
        }
        for b in range(Q):
            m[f"sidx{b}"] = _wrap16(srcidx[c][b])
            m[f"dloc{b}"] = _slotmajor(dstloc[c][b]).astype(_BF16)
        in_maps.append(m)
    return in_maps


def build_all(inputs, cfg=None):
    cfg = _derive(cfg or DEFAULT_CFG)
    sched = _build_schedule(inputs["src"], inputs["dst"], cfg)
    nc = _build_nc(cfg, sched[2])
    in_maps = _prepare_inputs(inputs, cfg, sched)
    return nc, in_maps, cfg


def kernel(**inputs):
    import concourse.bass_utils as _bu
    _bu.upload_artifacts = lambda tmpdir: f"file://{tmpdir}"  # offline container
    nc, in_maps, cfg = build_all(inputs)
    nc.compile()
    res = run_bass_kernel_spmd(nc, in_maps, list(range(NCORES)))
    out = np.concatenate([res.results[c]["out"] for c in range(NCORES)], axis=0)
    return out.astype(np.float32)

